# revision 1
# baseline (speedup 1.0000x reference)
"""Trainium2 Bass kernel for BasisAffinityGAT (8-core data-parallel over batch).

Computation per batch b:
  fused = concat(desc, nv) @ fusion_w.T + fusion_b          [N, D]
  q_k = l2norm(fused @ W_q[k]); k_k = l2norm(fused @ W_k[k])
  alpha[b,k] = softmax(q_k @ k_k.T / sqrt(D))               [K, N, N]
Outputs: (bias_log, alpha) where bias_log = log(max(0.01*mean_b(alpha), 1e-6))
broadcast over b.

Device strategy: batch sharded 4-per-core across 8 cores; weights replicated.
The host pre-casts all large inputs to bf16 (halves HBM traffic; the
normalization cancels most of the rounding, measured ~7e-5 rel err). All
activations kept transposed ([feature, token]) so every matmul contraction
runs over the partition dim with natural-layout weights as the stationary
operand; projections are re-cast to bf16 on the PSUM->SBUF copy so the small
logits matmuls run at bf16 rate. L2
normalization is folded into the logits via an outer-product of inverse norms
(one-hot ones-matmul partition reduction + ln/exp on ACT, with 1/sqrt(D)
folded into the exp bias). The softmax skips max-subtraction (logits are
cosines / sqrt(D), bounded by ~0.05). The mean over batch is finished on the
host from per-core partial sums.
"""

import math
import os
import sys

import numpy as np

# The kernel executes through jax's axon PJRT backend; a JAX_PLATFORMS=cpu
# pin (common for running the jax reference) would hide the NeuronCores.
# Clear it before jax initializes so platform auto-discovery finds axon.
if "axon" not in os.environ.get("JAX_PLATFORMS", "axon"):
    os.environ.pop("JAX_PLATFORMS", None)

try:  # the container puts the repo on sys.path; fall back to /opt otherwise
    import concourse  # noqa: F401
except ImportError:  # pragma: no cover
    sys.path.insert(0, "/opt/trn_rl_repo")

import concourse.tile as tile  # noqa: E402
from concourse import bacc, masks, mybir  # noqa: E402
from concourse.bass_utils import run_bass_kernel_spmd  # noqa: E402

B, N, D, K = 32, 128, 512, 8
CORES = 8
BL = B // CORES          # local batch per core
DC = D // 128            # 4 chunks of the feature/contraction dims
CC = 2 * D // 128        # 8 chunks of the concat dim
MOMENTUM = 0.99
EPS = 1e-6

F32 = mybir.dt.float32
F32R = mybir.dt.float32r
BF16 = mybir.dt.bfloat16
AF = mybir.ActivationFunctionType
ALU = mybir.AluOpType
AX = mybir.AxisListType

BN = BL * N              # 512: free dim packing all local batches


def build_kernel():
    nc = bacc.Bacc(
        "TRN2",
        target_bir_lowering=False,
        debug=False,
        enable_asserts=False,
    )

    desc = nc.dram_tensor("desc", [BL, N, D], BF16, kind="ExternalInput").ap()
    nv = nc.dram_tensor("nv", [BL, N, D], BF16, kind="ExternalInput").ap()
    wq = nc.dram_tensor("wq", [K, D, D], BF16, kind="ExternalInput").ap()
    wk = nc.dram_tensor("wk", [K, D, D], BF16, kind="ExternalInput").ap()
    fw = nc.dram_tensor("fw", [D, 2 * D], BF16, kind="ExternalInput").ap()
    fb = nc.dram_tensor("fb", [D], F32, kind="ExternalInput").ap()
    alpha_out = nc.dram_tensor(
        "alpha_out", [K, N, BL, N], F32, kind="ExternalOutput"
    ).ap()

    with tile.TileContext(nc) as tc:
        _emit(tc, desc, nv, wq, wk, fw, fb, alpha_out)
    nc.finalize()
    return nc


def _mm_f32r(nc, out, lhsT, rhs, **kw):
    nc.tensor.matmul(out, lhsT.bitcast(F32R), rhs.bitcast(F32R), **kw)


def _emit(tc, desc, nv, wq, wk, fw, fb, alpha_out):
    nc = tc.nc

    from contextlib import ExitStack

    ctx = ExitStack()
    with ctx:
        const_pool = ctx.enter_context(tc.tile_pool(name="const", bufs=1))
        fused_pool = ctx.enter_context(tc.tile_pool(name="fused", bufs=1))
        w_pool = ctx.enter_context(tc.tile_pool(name="w", bufs=2))
        qk_pool = ctx.enter_context(tc.tile_pool(name="qk", bufs=6))
        sq_pool = ctx.enter_context(tc.tile_pool(name="sq", bufs=3))
        sm_pool = ctx.enter_context(tc.tile_pool(name="sm", bufs=2))
        proj_ps = ctx.enter_context(tc.tile_pool(name="proj_ps", bufs=4, space="PSUM"))
        n2_ps_pool = ctx.enter_context(tc.tile_pool(name="n2_ps", bufs=1, space="PSUM"))
        lg_ps_pool = ctx.enter_context(tc.tile_pool(name="lg_ps", bufs=3, space="PSUM"))

        # --- constants -----------------------------------------------------
        # Column selectors for the norm ones-matmul: q sums land on psum row 0,
        # k sums on row 32 (both legal base partitions for later reads). The
        # middle columns are 1s so rows 1..31 hold junk > 0, keeping the
        # whole-tile Ln finite; those rows are never read.
        oh_q = const_pool.tile([128, 33], BF16)
        nc.vector.memset(oh_q[:], 1.0)
        nc.vector.memset(oh_q[:, 32:33], 0.0)
        oh_k = const_pool.tile([128, 33], BF16)
        nc.vector.memset(oh_k[:], 1.0)
        nc.vector.memset(oh_k[:, 0:1], 0.0)

        # q-side exp bias: folds the softmax 1/sqrt(D) into the inverse norm
        biasq = const_pool.tile([1, 1], F32)
        nc.vector.memset(biasq[:], -0.5 * math.log(D))

        ident = const_pool.tile([128, 128], BF16)
        masks.make_identity(nc, ident[:])

        # fusion bias as per-partition columns, one per output feature chunk
        fb_sb = const_pool.tile([128, DC], F32)
        nc.sync.dma_start(fb_sb[:], fb.rearrange("(c p) -> p c", p=128))

        # --- load + transpose inputs --------------------------------------
        # concatT[:, cc*BN + b*128 + n] = concat(desc, nv)[b, n, cc-chunk].T
        with tc.tile_pool(name="prep", bufs=1) as prep_pool, tc.tile_pool(
            name="io", bufs=2
        ) as io_pool:
            concatT = prep_pool.tile([128, CC * BN], BF16)
            concatT_v = concatT.rearrange("p (c w) -> p c w", w=BN)
            for t, src in ((0, desc), (1, nv)):
                ld = io_pool.tile([128, BL * D], BF16, tag="ld")
                for b in range(BL):
                    nc.sync.dma_start(ld[:, b * D : (b + 1) * D], src[b])
                    tp = proj_ps.tile([128, BN], BF16, tag="proj")
                    for c in range(DC):
                        nc.tensor.transpose(
                            tp[:, c * 128 : (c + 1) * 128],
                            ld[:, b * D + c * 128 : b * D + (c + 1) * 128],
                            ident[:],
                        )
                    nc.vector.tensor_copy(
                        concatT_v[:, t * DC : (t + 1) * DC, b * 128 : (b + 1) * 128],
                        tp.rearrange("p (c w) -> p c w", w=128),
                    )

            # fwT[:, c*D + i*128 + f] = fusion_w[i-chunk f, c-chunk].T
            fwT = prep_pool.tile([128, CC * D], BF16)
            fwT_v = fwT.rearrange("p (c w) -> p c w", w=D)
            fwb = prep_pool.tile([128, DC * 2 * D], BF16)
            for i in range(DC):
                nc.sync.dma_start(
                    fwb[:, i * 2 * D : (i + 1) * 2 * D],
                    fw[i * 128 : (i + 1) * 128, :],
                )
            for i in range(DC):
                for half in range(2):
                    tp = proj_ps.tile([128, BN], BF16, tag="proj")
                    for c in range(DC):
                        nc.tensor.transpose(
                            tp[:, c * 128 : (c + 1) * 128],
                            fwb[
                                :,
                                i * 2 * D
                                + half * D
                                + c * 128 : i * 2 * D
                                + half * D
                                + (c + 1) * 128,
                            ],
                            ident[:],
                        )
                    nc.scalar.activation(
                        fwT_v[
                            :, half * DC : (half + 1) * DC, i * 128 : (i + 1) * 128
                        ],
                        tp.rearrange("p (c w) -> p c w", w=128),
                        AF.Identity,
                    )

            # --- fusedT ----------------------------------------------------
            # fusedT[f, (b n)] = sum_c fusion_w[f, c] * concatT[c, (b n)] + fb[f]
            fusedT = fused_pool.tile([128, DC * BN], BF16)
            for f in range(DC):
                ft_ps = proj_ps.tile([128, BN], F32, tag="proj")
                for c in range(CC):
                    nc.tensor.matmul(
                        ft_ps[:],
                        fwT[:, c * D + f * 128 : c * D + (f + 1) * 128],
                        concatT[:, c * BN : (c + 1) * BN],
                        start=(c == 0),
                        stop=(c == CC - 1),
                    )
                nc.vector.tensor_scalar_add(
                    fusedT[:, f * BN : (f + 1) * BN],
                    ft_ps[:],
                    fb_sb[:, f : f + 1],
                )

        # --- per-basis pipeline, in groups of GRP bases --------------------
        # The Ln / Exp of the inverse-norm computation are batched per group
        # so the ACT table only swaps between the exp and ln sets once per
        # group instead of twice per basis (a table load costs ~2.7us).
        groups = [range(0, 5), range(5, 8)]
        for g, bases in enumerate(groups):
            GRP = len(bases)
            # ln of the squared norms, collected per group so the ACT table
            # only swaps exp->ln->exp once per group (Copy/Identity live in
            # every table set, so the interleaved copies don't add swaps)
            lng = sm_pool.tile([33, GRP * BN], F32, tag="lng", bufs=1)
            qsbs, ksbs = {}, {}
            for jr, j in enumerate(bases):
                # stream this basis' weights as plain f32, one DMA each
                wq_sb = w_pool.tile([128, DC * D], BF16, tag="wq")
                wk_sb = w_pool.tile([128, DC * D], BF16, tag="wk")
                for w_sb, w_dram in ((wq_sb, wq), (wk_sb, wk)):
                    nc.sync.dma_start(
                        w_sb.rearrange("p (d f) -> p d f", f=D),
                        w_dram[j].rearrange("(d p) f -> p d f", p=128),
                    )

                # projections: qT[f, (b n)] = sum_d Wq[d, f] fusedT[d, (b n)]
                qsb = qk_pool.tile([128, DC * BN], BF16, tag="q")
                ksb = qk_pool.tile([128, DC * BN], BF16, tag="k")
                qsbs[j], ksbs[j] = qsb, ksb
                for f in range(DC):
                    for proj_i, (w_sb, out_sb) in enumerate(
                        ((wq_sb, qsb), (wk_sb, ksb))
                    ):
                        pps = proj_ps.tile([128, BN], F32, tag="proj")
                        for d in range(DC):
                            nc.tensor.matmul(
                                pps[:],
                                w_sb[:, d * D + f * 128 : d * D + (f + 1) * 128],
                                fusedT[:, d * BN : (d + 1) * BN],
                                start=(d == 0),
                                stop=(d == DC - 1),
                            )
                        dst = out_sb[:, f * BN : (f + 1) * BN]
                        # PSUM -> SBUF move with bf16 cast, split ACT / DVE.
                        # In the last group DVE is congested (its passA
                        # overlaps the previous group's sc burst), so route
                        # half the k-copies to ACT there as well.
                        if proj_i == 0 or (g == len(groups) - 1 and f % 2 == 0):
                            nc.scalar.activation(dst, pps[:], AF.Copy)
                        else:
                            nc.vector.tensor_copy(dst, pps[:])

                # squared projections (bf16), tree-summed over the four
                # feature chunks on DVE, then a single one-hot ones-matmul
                # per projection sums over the partition (feature) dim into
                # n2 rows 0 (q) / 32 (k)
                n2 = n2_ps_pool.tile([33, BN], F32, tag="n2")
                for proj_i, psb in enumerate((qsb, ksb)):
                    sq = sq_pool.tile([128, DC * BN], BF16, tag="sq")
                    nc.vector.tensor_mul(sq[:], psb[:], psb[:])
                    h1 = sq_pool.tile([128, BN], BF16, tag="h1")
                    nc.vector.tensor_add(h1[:], sq[:, 0:BN], sq[:, BN : 2 * BN])
                    h2 = sq_pool.tile([128, BN], BF16, tag="h2")
                    nc.vector.tensor_add(
                        h2[:], sq[:, 2 * BN : 3 * BN], sq[:, 3 * BN : 4 * BN]
                    )
                    ssq = sq_pool.tile([128, BN], BF16, tag="ssq")
                    nc.vector.tensor_add(ssq[:], h1[:], h2[:])
                    nc.tensor.matmul(
                        n2[:],
                        oh_q[:] if proj_i == 0 else oh_k[:],
                        ssq[:],
                        start=(proj_i == 0),
                        stop=(proj_i == 1),
                    )
                nc.scalar.activation(
                    lng[:, jr * BN : (jr + 1) * BN], n2[:], AF.Ln
                )

            # inverse norms for the whole group:
            # inv = exp(-0.5 * ln(n2) + bias); the q side also carries the
            # 1/sqrt(D) softmax scale via its bias
            # pass B, two sub-loops: ACT runs in-order, so emit all the
            # inverse-norm exps / logits / outer / psum-freeing sc first --
            # otherwise each basis' exps queue behind the previous basis'
            # full softmax chain and the tail serializes.
            scs = {}
            for jr, j in enumerate(bases):
                qsb, ksb = qsbs[j], ksbs[j]
                jbs = slice(jr * BN, (jr + 1) * BN)
                # separate q/k tiles: matmul operands must share base
                # partition 0, so rows 0/32 of one tile cannot pair up
                invq = sm_pool.tile([1, BN], BF16, tag="invq", bufs=4)
                nc.scalar.activation(
                    invq[:], lng[0:1, jbs], AF.Exp, bias=biasq[:], scale=-0.5
                )
                invk = sm_pool.tile([1, BN], BF16, tag="invk", bufs=4)
                nc.scalar.activation(invk[:], lng[32:33, jbs], AF.Exp, scale=-0.5)
                # logits and outer-product of inverse norms, all b packed
                lg = lg_ps_pool.tile([128, BN], F32, tag="lg")
                ou = lg_ps_pool.tile([128, BN], F32, tag="lg")
                for b in range(BL):
                    bs = slice(b * 128, (b + 1) * 128)
                    for f in range(DC):
                        nc.tensor.matmul(
                            lg[:, bs],
                            qsb[:, f * BN + b * 128 : f * BN + (b + 1) * 128],
                            ksb[:, f * BN + b * 128 : f * BN + (b + 1) * 128],
                            start=(f == 0),
                            stop=(f == DC - 1),
                        )
                    nc.tensor.matmul(
                        ou[:, bs], invq[:, bs], invk[:, bs], start=True, stop=True
                    )

                # softmax over m (free dim within each b block); logits are
                # cosine/sqrt(D), |x| <= 0.05, so no max-subtraction needed
                ou_sb = sm_pool.tile([128, BN], F32, tag="ou_sb", bufs=3)
                nc.scalar.activation(ou_sb[:], ou[:], AF.Copy)
                sc = sm_pool.tile([128, BN], F32, tag="sc", bufs=6)
                nc.vector.tensor_mul(sc[:], lg[:], ou_sb[:])
                scs[j] = sc

            for jr, j in enumerate(bases):
                sc = scs[j]
                ex = sm_pool.tile([128, BN], F32, tag="ex")
                nc.scalar.activation(ex[:], sc[:], AF.Exp)
                den = sm_pool.tile([128, BL], F32, tag="den")
                nc.vector.tensor_reduce(
                    den[:], ex.rearrange("p (b m) -> p b m", m=N), axis=AX.X,
                    op=ALU.add,
                )
                rec = sm_pool.tile([128, BL], F32, tag="rec")
                nc.vector.reciprocal(rec[:], den[:])
                al = sm_pool.tile([128, BN], F32, tag="al")
                nc.vector.tensor_mul(
                    al.rearrange("p (b m) -> p b m", m=N),
                    ex.rearrange("p (b m) -> p b m", m=N),
                    rec[:, :, None].broadcast_to([128, BL, N]),
                )
                nc.sync.dma_start(alpha_out[j].rearrange("n b m -> n (b m)"), al[:])


_CACHE = {}


def _get_nc():
    if "nc" not in _CACHE:
        _CACHE["nc"] = build_kernel()
    return _CACHE["nc"]


def shard_inputs(desc_embeddings, name_value_embeddings, W_q, W_k, fusion_w, fusion_b):
    import ml_dtypes

    bf16 = ml_dtypes.bfloat16
    # pre-cast the big operands on the host: halves HBM traffic, and the
    # device pipeline computes in bf16 anyway
    full = {
        "wq": np.ascontiguousarray(np.asarray(W_q, dtype=np.float32).astype(bf16)),
        "wk": np.ascontiguousarray(np.asarray(W_k, dtype=np.float32).astype(bf16)),
        "fw": np.ascontiguousarray(np.asarray(fusion_w, dtype=np.float32).astype(bf16)),
        "fb": np.ascontiguousarray(fusion_b, dtype=np.float32),
    }
    desc_b = np.asarray(desc_embeddings, dtype=np.float32).astype(bf16)
    nv_b = np.asarray(name_value_embeddings, dtype=np.float32).astype(bf16)
    in_maps = []
    for c in range(CORES):
        sl = slice(c * BL, (c + 1) * BL)
        m = dict(full)
        m["desc"] = np.ascontiguousarray(desc_b[sl])
        m["nv"] = np.ascontiguousarray(nv_b[sl])
        in_maps.append(m)
    return in_maps


def assemble_outputs(results):
    alpha = np.empty((B, K, N, N), dtype=np.float32)
    asum = np.zeros((K, N, N), dtype=np.float32)
    for c, r in enumerate(results):
        # device layout [K, N, BL, N] -> [BL, K, N, N]
        alpha[c * BL : (c + 1) * BL] = np.transpose(r["alpha_out"], (2, 0, 1, 3))
        asum += r["alpha_out"].sum(axis=2)
    ema = np.float32(1.0 - MOMENTUM) * (asum / np.float32(B))
    bias_log = np.log(np.maximum(ema, np.float32(EPS)))
    bias_log = np.broadcast_to(bias_log[None], (B, K, N, N))
    return bias_log, alpha


def kernel(desc_embeddings, name_value_embeddings, W_q, W_k, fusion_w, fusion_b,
           _trace=False):
    nc = _get_nc()
    in_maps = shard_inputs(
        desc_embeddings, name_value_embeddings, W_q, W_k, fusion_w, fusion_b
    )
    res = run_bass_kernel_spmd(nc, in_maps, core_ids=list(range(CORES)), trace=_trace)
    out = assemble_outputs(res.results)
    if _trace:
        return out, res
    return out



# revision 17
# speedup vs baseline: 1.6091x; 1.6091x over previous
"""Trainium2 Bass kernel for BasisAffinityGAT (8-core data-parallel over batch).

Computation per batch b:
  fused = concat(desc, nv) @ fusion_w.T + fusion_b          [N, D]
  q_k = l2norm(fused @ W_q[k]); k_k = l2norm(fused @ W_k[k])
  alpha[b,k] = softmax(q_k @ k_k.T / sqrt(D))               [K, N, N]
Outputs: (bias_log, alpha); bias_log = log(max(0.01*mean_b(alpha), 1e-6)).

Strategy (v2, fp8 DoubleRow):
- Batch sharded 4-per-core; all matmul operands cast to fp8-e4m3 on the host
  in pre-transposed, chunk-major layouts so the device does ZERO transposes.
- Every contraction >=256 runs as fp8 DoubleRow matmuls (2 fp8/cell: pairs of
  128-row chunks contract per instruction).
- Projections land in PSUM f32 and are evacuated to SBUF fp8 through three
  lanes in parallel (casting SWDGE DMA on gpsimd, ACT copy, DVE copy).
- Norms: per-token sum(q^2) via elementwise squares + one-hot DoubleRow
  matmuls that scatter each basis' q/k norm rows into a shared [16,512] PSUM
  tile per group of 4 bases; one DVE reciprocal + one ACT sqrt per group,
  then one SBUF->SBUF DMA flattens inverse norms onto partition 0 so they can
  feed rank-1 outer-product matmuls.
- Softmax: logits (fp8 DR) * outer(invq, invk) on DVE, exp on ACT with the
  1/sqrt(D) folded into the activation scale. Unnormalized exp ships out in
  bf16; the host divides by row sums (and finishes the EMA mean), which is
  exactly softmax.
All fp8 rounding lands either ahead of the l2 normalization (which cancels
scale errors) or on cosine-scaled logits |x|<=0.05, keeping rel err ~1e-3.
"""

import math
import os
import sys

import numpy as np

if "axon" not in os.environ.get("JAX_PLATFORMS", "axon"):
    os.environ.pop("JAX_PLATFORMS", None)

try:
    import concourse  # noqa: F401
except ImportError:  # pragma: no cover
    sys.path.insert(0, "/opt/trn_rl_repo")

import concourse.tile as tile  # noqa: E402
from concourse import bacc, mybir  # noqa: E402
from concourse.bass_utils import run_bass_kernel_spmd  # noqa: E402

B, N, D, K = 32, 128, 512, 8
CORES = 8
BL = B // CORES          # 4 local batches
BN = BL * N              # 512 tokens per core
DC = D // 128            # 4 feature chunks
CC = 2 * D // 128        # 8 concat chunks
GRP = 2                  # bases per norm group
MOMENTUM = 0.99
EPS = 1e-6

F32 = mybir.dt.float32
BF16 = mybir.dt.bfloat16
FP8 = mybir.dt.float8e4
AF = mybir.ActivationFunctionType
PM = mybir.MatmulPerfMode

EXP_SCALE = 1.0 / math.sqrt(D)

# engine lane tables: copies per quarter-projection [side][fc],
# squares per half [side][half]
COPY_LANE = [["act", "act", "act", "act"], ["dve", "dve", "dve", "act"]]
SQ_LANE = [["pool", "dve"], ["pool", "act"]]


def build_kernel():
    nc = bacc.Bacc(
        "TRN2",
        target_bir_lowering=False,
        debug=False,
        enable_asserts=False,
    )

    concatT = nc.dram_tensor("concatT", [128, CC * BN], FP8, kind="ExternalInput").ap()
    fwT = nc.dram_tensor("fwT", [128, CC * D], FP8, kind="ExternalInput").ap()
    wqk = nc.dram_tensor("wqk", [K, 128, 2 * DC * D], FP8, kind="ExternalInput").ap()
    oh = nc.dram_tensor("oh", [128, 2 * GRP * 32], FP8, kind="ExternalInput").ap()
    fb = nc.dram_tensor("fb", [128, DC], F32, kind="ExternalInput").ap()
    ex_out = nc.dram_tensor("ex_out", [K, N, BN], BF16, kind="ExternalOutput").ap()

    with tile.TileContext(nc) as tc:
        _emit(tc, concatT, fwT, wqk, oh, fb, ex_out)
    nc.finalize()
    return nc


def _dr(nc, out, lhsT, rhs, start, stop):
    nc.tensor.matmul(
        out, lhsT, rhs, start=start, stop=stop,
        perf_mode=PM.DoubleRow, skip_group_check=True,
    )


def _emit(tc, concatT, fwT, wqk, oh, fb, ex_out):
    nc = tc.nc
    from contextlib import ExitStack

    ctx = ExitStack()
    with ctx:
        const_pool = ctx.enter_context(tc.tile_pool(name="const", bufs=1))
        fused_pool = ctx.enter_context(tc.tile_pool(name="fused", bufs=1))
        w_pool = ctx.enter_context(tc.tile_pool(name="w", bufs=3))
        qk_pool = ctx.enter_context(tc.tile_pool(name="qk", bufs=8))
        sq_pool = ctx.enter_context(tc.tile_pool(name="sq", bufs=4))
        inv_pool = ctx.enter_context(tc.tile_pool(name="inv", bufs=2))
        sm_pool = ctx.enter_context(tc.tile_pool(name="sm", bufs=3))
        proj_ps = ctx.enter_context(tc.tile_pool(name="proj_ps", bufs=4, space="PSUM"))
        n2_ps = ctx.enter_context(tc.tile_pool(name="n2_ps", bufs=2, space="PSUM"))
        lg_ps = ctx.enter_context(tc.tile_pool(name="lg_ps", bufs=1, space="PSUM"))
        ou_ps = ctx.enter_context(tc.tile_pool(name="ou_ps", bufs=1, space="PSUM"))

        # ---- constants ---------------------------------------------------
        # tiny dummy Sqrt pins the {sqrt, square, copy, identity} ACT table
        # set once at t=0; every later activation stays in-set (no reloads)
        dummy = const_pool.tile([1, 1], F32)
        nc.vector.memset(dummy[:], 1.0)
        nc.scalar.activation(dummy[:], dummy[:], AF.Sqrt)
        oh_sb = const_pool.tile([128, 2 * GRP * 32], FP8)
        nc.sync.dma_start(oh_sb[:], oh)
        fb_sb = const_pool.tile([128, DC], F32)
        nc.sync.dma_start(fb_sb[:], fb)

        # ---- fused = concat @ fw.T + fb, in fp8 transposed layout --------
        # fusedT[p, f, t] over f-chunks; contraction over CC=8 chunks as 4
        # DoubleRow pairs.
        fusedT = fused_pool.tile([128, DC * BN], FP8)
        fusedT_v = fusedT.rearrange("p (c t) -> p c t", t=BN)
        with tc.tile_pool(name="prep", bufs=1) as prep_pool:
            ccT = prep_pool.tile([128, CC * BN], FP8)
            nc.sync.dma_start(ccT[:], concatT)
            fwT_sb = prep_pool.tile([128, CC * D], FP8)
            nc.sync.dma_start(fwT_sb[:], fwT)
            ccT_v = ccT.rearrange("p (c t) -> p c t", t=BN)
            fwT_v = fwT_sb.rearrange("p (c f) -> p c f", f=D)
            for f in range(DC):
                fps = proj_ps.tile([128, BN], F32, tag="pp", name="fps")
                for pr in range(CC // 2):
                    _dr(
                        nc, fps[:],
                        fwT_v[:, 2 * pr : 2 * pr + 2, f * 128 : (f + 1) * 128],
                        ccT_v[:, 2 * pr : 2 * pr + 2, :],
                        start=(pr == 0), stop=(pr == CC // 2 - 1),
                    )
                if f % 2 == 0:
                    nc.scalar.activation(
                        fusedT_v[:, f, :], fps[:],
                        AF.Identity, bias=fb_sb[:, f : f + 1],
                    )
                else:
                    nc.vector.tensor_scalar_add(
                        fusedT_v[:, f, :], fps[:], fb_sb[:, f : f + 1],
                    )

        fusedT_p = fusedT.rearrange("p (c t) -> p c t", t=BN)

        # ---- per-basis production + grouped norm / softmax tail ----------
        # Per group of 4 bases, n2[16, BN] collects row jg (q) / 4+jg (k).
        n2_tiles = {}
        qsbs, ksbs = {}, {}
        sq_tiles = {}

        def norm_mm(j):
            # one-hot DoubleRow matmuls into n2 rows (row jg for q, 4+jg
            # for k; rows 8..15 take zeros). Emitted one basis late so PE
            # never head-of-line blocks on the squares.
            g, jg = divmod(j, GRP)
            n2 = n2_tiles[g]
            for side in range(2):
                sqt_v = sq_tiles.pop((j, side))
                blk = (jg * 2 + side) * 32
                ohv = oh_sb[:, blk : blk + 32].rearrange(
                    "p (i c) -> p i c", i=2
                )
                first = jg == 0 and side == 0
                last = jg == GRP - 1 and side == 1
                for pr in range(2):
                    _dr(
                        nc, n2[:],
                        ohv[:],
                        sqt_v[:, 2 * pr : 2 * pr + 2, :],
                        start=(first and pr == 0), stop=(last and pr == 1),
                    )

        def produce(j):
            g, jg = divmod(j, GRP)
            wqk_sb = w_pool.tile([128, 2 * DC * D], FP8, tag="wqk")
            nc.sync.dma_start(wqk_sb[:], wqk[j])
            wq_sb = wqk_sb[:, : DC * D]
            wk_sb = wqk_sb[:, DC * D :]

            if jg == 0:
                n2_tiles[g] = n2_ps.tile([16, BN], F32, tag="n2", name="n2")
            n2 = n2_tiles[g]

            sq_jobs = []
            for side, (w_sb, store) in enumerate(
                ((wq_sb, qsbs), (wk_sb, ksbs))
            ):
                w_v = w_sb.rearrange("p (c f) -> p c f", f=D)
                psb = qk_pool.tile([128, DC * BN], FP8, tag="qk" + str(side), name="psb")
                store[j] = psb
                sqt = sq_pool.tile([128, DC * BN], FP8, tag="sq" + str(side), name="sqt")
                psb_v = psb.rearrange("p (c t) -> p c t", t=BN)
                sqt_v = sqt.rearrange("p (c t) -> p c t", t=BN)
                for fc in range(DC):
                    pp = proj_ps.tile([128, BN], F32, tag="pp")
                    for pr in range(2):
                        _dr(
                            nc, pp[:],
                            w_v[:, 2 * pr : 2 * pr + 2, fc * 128 : (fc + 1) * 128],
                            fusedT_p[:, 2 * pr : 2 * pr + 2, :],
                            start=(pr == 0), stop=(pr == 1),
                        )
                    # PSUM evacuation: the copy is the sole pp reader, so
                    # the PE pipeline only waits on one ACT/DVE op per
                    # quarter-projection (ring depth 4).
                    dst = psb_v[:, fc, :]
                    if COPY_LANE[side][fc] == "act":
                        nc.scalar.activation(dst, pp[:], AF.Copy)
                    else:
                        nc.vector.tensor_copy(dst, pp[:])
                    sq_jobs.append((side, fc, sqt_v, psb_v))
                sq_tiles[(j, side)] = sqt_v
            # squares read the SBUF fp8 copies (off the PE critical loop),
            # emitted as halves to keep op counts down
            for side, fc, sqt_v, psb_v in sq_jobs:
                if fc % 2:
                    continue
                sdst = sqt_v[:, fc : fc + 2, :]
                ssrc = psb_v[:, fc : fc + 2, :]
                sq_lane = SQ_LANE[side][fc // 2]
                if sq_lane == "act":
                    nc.scalar.activation(sdst, ssrc, AF.Square)
                elif sq_lane == "dve":
                    nc.vector.tensor_mul(sdst, ssrc, ssrc)
                else:
                    nc.gpsimd.tensor_mul(sdst, ssrc, ssrc)

        invfs = {}

        def finish_group(g):
            n2 = n2_tiles[g]
            rsb = inv_pool.tile([2 * GRP, BN], F32, tag="rsb")
            nc.vector.reciprocal(rsb[:], n2[0 : 2 * GRP, :])
            inv = inv_pool.tile([2 * GRP, BN], BF16, tag="inv")
            nc.scalar.activation(inv[:], rsb[:], AF.Sqrt)
            invf = inv_pool.tile([1, 2 * GRP * BN], BF16, tag="invf")
            nc.sync.dma_start(invf[:], inv[:])
            invfs[g] = invf

        def tail(j):
            g, jg = divmod(j, GRP)
            invf = invfs[g]
            qsb = qsbs.pop(j).rearrange("p (c t) -> p c t", t=BN)
            ksb = ksbs.pop(j).rearrange("p (c t) -> p c t", t=BN)
            lg = lg_ps.tile([128, BN], F32, tag="lg")
            ou = ou_ps.tile([128, BN], F32, tag="ou")
            for b in range(BL):
                bs = slice(b * 128, (b + 1) * 128)
                for pr in range(2):
                    _dr(
                        nc, lg[:, bs],
                        qsb[:, 2 * pr : 2 * pr + 2, bs],
                        ksb[:, 2 * pr : 2 * pr + 2, bs],
                        start=(pr == 0), stop=(pr == 1),
                    )
                qoff = jg * BN + b * 128
                koff = GRP * BN + jg * BN + b * 128
                nc.tensor.matmul(
                    ou[:, bs],
                    invf[:, qoff : qoff + 128],
                    invf[:, koff : koff + 128],
                    start=True, stop=True, skip_group_check=True,
                )
            # DVE may read only one PSUM operand; stage ou in SBUF with the
            # softmax 1/sqrt(D) folded into the copy scale
            ou_sb = sm_pool.tile([128, BN], BF16, tag="ou_sb")
            nc.scalar.activation(ou_sb[:], ou[:], AF.Copy, scale=EXP_SCALE)
            scb = sm_pool.tile([128, BN], BF16, tag="scb")
            nc.vector.tensor_mul(scb[:], lg[:], ou_sb[:])
            # y = x + x^2/2 = exp(x)-1 to O(x^3); the host adds the 1 and
            # normalizes (|x| <= 0.05 so rel err ~ 2e-5)
            yb = sm_pool.tile([128, BN], BF16, tag="yb")
            acc = sm_pool.tile([128, 1], F32, tag="acc")
            nc.vector.affine_mul_reduce(
                yb[:], acc[:], scb[:], scb[:], scale=0.5, bias=1.0,
            )
            nc.sync.dma_start(ex_out[j].rearrange("n t -> n t"), yb[:])

        # software-pipelined schedule: norm matmuls lag production by one
        # basis; tails start as soon as their group's inverse norms exist.
        ablate = int(os.environ.get("KABLATE", "5"))
        if ablate < 5:
            if ablate >= 1:
                for j in range(K):
                    produce(j)
            if ablate >= 2:
                for j in range(K):
                    norm_mm(j)
            if ablate >= 3:
                for g in range(K // GRP):
                    finish_group(g)
            if ablate >= 4:
                for j in range(K):
                    tail(j)
            return
        produce(0)
        produce(1)
        norm_mm(0)
        produce(2)
        norm_mm(1)
        finish_group(0)
        produce(3)
        norm_mm(2)
        produce(4)
        norm_mm(3)
        finish_group(1)
        tail(0)
        produce(5)
        norm_mm(4)
        tail(1)
        produce(6)
        norm_mm(5)
        finish_group(2)
        tail(2)
        produce(7)
        norm_mm(6)
        tail(3)
        tail(4)
        norm_mm(7)
        finish_group(3)
        tail(5)
        tail(6)
        tail(7)


_CACHE = {}


def _get_nc():
    if "nc" not in _CACHE:
        _CACHE["nc"] = build_kernel()
    return _CACHE["nc"]


def shard_inputs(desc_embeddings, name_value_embeddings, W_q, W_k, fusion_w, fusion_b):
    import ml_dtypes

    fp8 = ml_dtypes.float8_e4m3

    def to8(x):
        return np.ascontiguousarray(
            np.clip(np.asarray(x, dtype=np.float32), -224.0, 224.0).astype(fp8)
        )

    # fwT[p, ch, f] = fusion_w[f, ch*128+p]
    fwT = to8(
        np.asarray(fusion_w, np.float32)
        .T.reshape(CC, 128, D)
        .transpose(1, 0, 2)
        .reshape(128, CC * D)
    )
    # wq[j, p, dch, f] = W[j, dch*128+p, f]
    def wprep(W):
        return to8(
            np.asarray(W, np.float32)
            .reshape(K, DC, 128, D)
            .transpose(0, 2, 1, 3)
            .reshape(K, 128, DC * D)
        )

    wqk8 = np.concatenate([wprep(W_q), wprep(W_k)], axis=2)

    # one-hot scatter weights for the norm reduce
    ohnp = np.zeros((128, 2 * GRP * 32), np.float32)
    for jg in range(GRP):
        for s in range(2):
            blk = (jg * 2 + s) * 32
            tgt = jg + GRP * s
            ohnp[:, blk + tgt] = 1.0       # i = 0
            ohnp[:, blk + 16 + tgt] = 1.0  # i = 1
    oh8 = to8(ohnp)

    fb128 = np.ascontiguousarray(
        np.asarray(fusion_b, np.float32).reshape(DC, 128).T
    )

    desc = np.asarray(desc_embeddings, np.float32)
    nv = np.asarray(name_value_embeddings, np.float32)
    in_maps = []
    for c in range(CORES):
        dloc = desc[c * BL : (c + 1) * BL]      # [BL, N, D]
        nloc = nv[c * BL : (c + 1) * BL]
        # concatT[p, ch, b*128+n] = concat[b, n, ch*128+?]: chunks 0-3 desc,
        # 4-7 nv; within chunk partition p = feature ch*128+p
        cat = np.concatenate([dloc, nloc], axis=2)        # [BL, N, 2D]
        cT = (
            cat.reshape(BL * N, CC, 128)
            .transpose(2, 1, 0)                           # [128, CC, BL*N]
            .reshape(128, CC * BN)
        )
        in_maps.append(
            {
                "concatT": to8(cT),
                "fwT": fwT,
                "wqk": wqk8,
                "oh": oh8,
                "fb": fb128,
            }
        )
    return in_maps


def assemble_outputs(results):
    alpha = np.empty((B, K, N, N), dtype=np.float32)
    asum = np.zeros((K, N, N), dtype=np.float32)
    for c, r in enumerate(results):
        y = np.asarray(r["ex_out"], dtype=np.float32)     # [K, N, BL*N]
        y = y.reshape(K, N, BL, N)
        den = y.sum(axis=3, keepdims=True) + np.float32(N)
        al = (1.0 + y) / den                              # [K, N, BL, N]
        al = al.transpose(2, 0, 1, 3)                     # [BL, K, N, N]
        alpha[c * BL : (c + 1) * BL] = al
        asum += al.sum(axis=0)
    ema = np.float32(1.0 - MOMENTUM) * (asum / np.float32(B))
    bias_log = np.log(np.maximum(ema, np.float32(EPS)))
    bias_log = np.broadcast_to(bias_log[None], (B, K, N, N))
    return bias_log, alpha


def kernel(desc_embeddings, name_value_embeddings, W_q, W_k, fusion_w, fusion_b,
           _trace=False):
    nc = _get_nc()
    in_maps = shard_inputs(
        desc_embeddings, name_value_embeddings, W_q, W_k, fusion_w, fusion_b
    )
    res = run_bass_kernel_spmd(nc, in_maps, core_ids=list(range(CORES)), trace=_trace)
    out = assemble_outputs(res.results)
    if _trace:
        return out, res
    return out


# revision 24
# speedup vs baseline: 1.6479x; 1.0241x over previous
"""Trainium2 Bass kernel for BasisAffinityGAT (8-core data-parallel over batch).

Computation per batch b:
  fused = concat(desc, nv) @ fusion_w.T + fusion_b          [N, D]
  q_k = l2norm(fused @ W_q[k]); k_k = l2norm(fused @ W_k[k])
  alpha[b,k] = softmax(q_k @ k_k.T / sqrt(D))               [K, N, N]
Outputs: (bias_log, alpha); bias_log = log(max(0.01*mean_b(alpha), 1e-6)).

Strategy (v2, fp8 DoubleRow):
- Batch sharded 4-per-core; all matmul operands cast to fp8-e4m3 on the host
  in pre-transposed, chunk-major layouts so the device does ZERO transposes.
- Every contraction >=256 runs as fp8 DoubleRow matmuls (2 fp8/cell: pairs of
  128-row chunks contract per instruction).
- Projections land in PSUM f32 and are evacuated to SBUF fp8 through three
  lanes in parallel (casting SWDGE DMA on gpsimd, ACT copy, DVE copy).
- Norms: per-token sum(q^2) via elementwise squares + one-hot DoubleRow
  matmuls that scatter each basis' q/k norm rows into a shared [16,512] PSUM
  tile per group of 4 bases; one DVE reciprocal + one ACT sqrt per group,
  then one SBUF->SBUF DMA flattens inverse norms onto partition 0 so they can
  feed rank-1 outer-product matmuls.
- Softmax: logits (fp8 DR) * outer(invq, invk) on DVE, exp on ACT with the
  1/sqrt(D) folded into the activation scale. Unnormalized exp ships out in
  bf16; the host divides by row sums (and finishes the EMA mean), which is
  exactly softmax.
All fp8 rounding lands either ahead of the l2 normalization (which cancels
scale errors) or on cosine-scaled logits |x|<=0.05, keeping rel err ~1e-3.
"""

import math
import os
import sys

import numpy as np

if "axon" not in os.environ.get("JAX_PLATFORMS", "axon"):
    os.environ.pop("JAX_PLATFORMS", None)

try:
    import concourse  # noqa: F401
except ImportError:  # pragma: no cover
    sys.path.insert(0, "/opt/trn_rl_repo")

import concourse.tile as tile  # noqa: E402
from concourse import bacc, mybir  # noqa: E402
from concourse.bass_utils import run_bass_kernel_spmd  # noqa: E402

B, N, D, K = 32, 128, 512, 8
CORES = 8
BL = B // CORES          # 4 local batches
BN = BL * N              # 512 tokens per core
DC = D // 128            # 4 feature chunks
CC = 2 * D // 128        # 8 concat chunks
GROUPS = [(0, 1), (2, 3), (4, 5), (6,), (7,)]   # norm-group partition
GRP_OF = {j: g for g, grp in enumerate(GROUPS) for j in grp}
JG_OF = {j: grp.index(j) for grp in GROUPS for j in grp}
GRP = 2                  # max bases per norm group (oh layout block size)
MOMENTUM = 0.99
EPS = 1e-6

F32 = mybir.dt.float32
BF16 = mybir.dt.bfloat16
FP8 = mybir.dt.float8e4
AF = mybir.ActivationFunctionType
PM = mybir.MatmulPerfMode

EXP_SCALE = 1.0 / math.sqrt(D)

# engine lane tables: copies per half-projection [side], squares per
# half [side][half]; env-overridable for tuning sweeps
COPY_LANE = os.environ.get("KCOPY", "a,d").split(",")
COPY_LANE = [{"a": "act", "d": "dve"}[x] for x in COPY_LANE]
_SQ = os.environ.get("KSQ", "p,a,p,d").split(",")
_SQ = [{"a": "act", "d": "dve", "p": "pool"}[x] for x in _SQ]
SQ_LANE = [[_SQ[0], _SQ[1]], [_SQ[2], _SQ[3]]]


def build_kernel():
    nc = bacc.Bacc(
        "TRN2",
        target_bir_lowering=False,
        debug=False,
        enable_asserts=False,
    )

    concatT = nc.dram_tensor("concatT", [128, CC * BN], FP8, kind="ExternalInput").ap()
    fwT = nc.dram_tensor("fwT", [128, CC * D], FP8, kind="ExternalInput").ap()
    wqk = nc.dram_tensor("wqk", [K, 128, 2 * DC * D], FP8, kind="ExternalInput").ap()
    oh = nc.dram_tensor("oh", [128, 6 * 32], FP8, kind="ExternalInput").ap()
    fb = nc.dram_tensor("fb", [128, DC], F32, kind="ExternalInput").ap()
    ex_out = nc.dram_tensor("ex_out", [K, N, BN], BF16, kind="ExternalOutput").ap()

    with tile.TileContext(nc) as tc:
        _emit(tc, concatT, fwT, wqk, oh, fb, ex_out)
    nc.finalize()
    return nc


def _dr(nc, out, lhsT, rhs, start, stop):
    nc.tensor.matmul(
        out, lhsT, rhs, start=start, stop=stop,
        perf_mode=PM.DoubleRow, skip_group_check=True,
    )


def _emit(tc, concatT, fwT, wqk, oh, fb, ex_out):
    nc = tc.nc
    from contextlib import ExitStack

    ctx = ExitStack()
    with ctx:
        const_pool = ctx.enter_context(tc.tile_pool(name="const", bufs=1))
        fused_pool = ctx.enter_context(tc.tile_pool(name="fused", bufs=1))
        w_pool = ctx.enter_context(tc.tile_pool(name="w", bufs=3))
        qk_pool = ctx.enter_context(tc.tile_pool(name="qk", bufs=8))
        sq_pool = ctx.enter_context(tc.tile_pool(name="sq", bufs=4))
        inv_pool = ctx.enter_context(tc.tile_pool(name="inv", bufs=2))
        sm_pool = ctx.enter_context(tc.tile_pool(name="sm", bufs=3))
        proj_ps = ctx.enter_context(tc.tile_pool(name="proj_ps", bufs=1, space="PSUM"))
        n2_ps = ctx.enter_context(tc.tile_pool(name="n2_ps", bufs=2, space="PSUM"))
        lg_ps = ctx.enter_context(tc.tile_pool(name="lg_ps", bufs=1, space="PSUM"))
        ou_ps = ctx.enter_context(tc.tile_pool(name="ou_ps", bufs=1, space="PSUM"))

        # ---- constants ---------------------------------------------------
        # tiny dummy Sqrt pins the {sqrt, square, copy, identity} ACT table
        # set once at t=0; every later activation stays in-set (no reloads)
        dummy = const_pool.tile([1, 1], F32)
        nc.vector.memset(dummy[:], 1.0)
        nc.scalar.activation(dummy[:], dummy[:], AF.Sqrt)
        oh_sb = const_pool.tile([128, 6 * 32], FP8)
        nc.sync.dma_start(oh_sb[:], oh)
        fb_sb = const_pool.tile([128, DC], F32)
        nc.sync.dma_start(fb_sb[:], fb)

        # ---- fused = concat @ fw.T + fb, in fp8 transposed layout --------
        # fusedT[p, f, t] over f-chunks; contraction over CC=8 chunks as 4
        # DoubleRow pairs.
        fusedT = fused_pool.tile([128, DC * BN], FP8)
        fusedT_v = fusedT.rearrange("p (c t) -> p c t", t=BN)
        with tc.tile_pool(name="prep", bufs=1) as prep_pool:
            ccT = prep_pool.tile([128, CC * BN], FP8)
            fwT_sb = prep_pool.tile([128, CC * D], FP8)
            half = CC // 2
            nc.sync.dma_start(fwT_sb[:, : half * D], fwT[:, : half * D])
            nc.sync.dma_start(ccT[:, : half * BN], concatT[:, : half * BN])
            nc.sync.dma_start(fwT_sb[:, half * D :], fwT[:, half * D :])
            nc.sync.dma_start(ccT[:, half * BN :], concatT[:, half * BN :])
            ccT_v = ccT.rearrange("p (c t) -> p c t", t=BN)
            fwT_v = fwT_sb.rearrange("p (c f) -> p c f", f=D)
            for f in range(DC):
                fps = proj_ps.tile([128, BN], F32, tag="pp0", name="fps")
                for pr in range(CC // 2):
                    _dr(
                        nc, fps[:],
                        fwT_v[:, 2 * pr : 2 * pr + 2, f * 128 : (f + 1) * 128],
                        ccT_v[:, 2 * pr : 2 * pr + 2, :],
                        start=(pr == 0), stop=(pr == CC // 2 - 1),
                    )
                if f % 2 == 0:
                    nc.scalar.activation(
                        fusedT_v[:, f, :], fps[:],
                        AF.Identity, bias=fb_sb[:, f : f + 1],
                    )
                else:
                    nc.vector.tensor_scalar_add(
                        fusedT_v[:, f, :], fps[:], fb_sb[:, f : f + 1],
                    )

        fusedT_p = fusedT.rearrange("p (c t) -> p c t", t=BN)

        # ---- per-basis production + grouped norm / softmax tail ----------
        # Per group of 4 bases, n2[16, BN] collects row jg (q) / 4+jg (k).
        n2_tiles = {}
        qsbs, ksbs = {}, {}
        sq_tiles = {}

        def norm_mm(j):
            # one-hot DoubleRow matmuls into n2 rows (row jg for q, S+jg
            # for k; other rows take zeros). Emitted one basis late so PE
            # never head-of-line blocks on the squares.
            g, jg = GRP_OF[j], JG_OF[j]
            S = len(GROUPS[g])
            n2 = n2_tiles[g]
            for side in range(2):
                sqt_v = sq_tiles.pop((j, side))
                if S == 2:
                    blk = (jg * 2 + side) * 32
                else:
                    blk = (4 + side) * 32
                ohv = oh_sb[:, blk : blk + 32].rearrange(
                    "p (i c) -> p i c", i=2
                )
                first = jg == 0 and side == 0
                last = jg == S - 1 and side == 1
                for pr in range(2):
                    _dr(
                        nc, n2[:],
                        ohv[:],
                        sqt_v[:, 2 * pr : 2 * pr + 2, :],
                        start=(first and pr == 0), stop=(last and pr == 1),
                    )

        def produce(j):
            g, jg = GRP_OF[j], JG_OF[j]
            wqk_sb = w_pool.tile([128, 2 * DC * D], FP8, tag="wqk")
            nc.sync.dma_start(wqk_sb[:], wqk[j])
            wq_sb = wqk_sb[:, : DC * D]
            wk_sb = wqk_sb[:, DC * D :]

            if jg == 0:
                n2_tiles[g] = n2_ps.tile([16, BN], F32, tag="n2", name="n2")
            n2 = n2_tiles[g]
            del n2

            sq_jobs = []
            for side, (w_sb, store) in enumerate(
                ((wq_sb, qsbs), (wk_sb, ksbs))
            ):
                w_v = w_sb.rearrange("p (c f) -> p c f", f=D)
                psb = qk_pool.tile([128, DC * BN], FP8, tag="qk" + str(side), name="psb")
                store[j] = psb
                sqt = sq_pool.tile([128, DC * BN], FP8, tag="sq" + str(side), name="sqt")
                psb_v = psb.rearrange("p (c t) -> p c t", t=BN)
                sqt_v = sqt.rearrange("p (c t) -> p c t", t=BN)
                for half in range(2):
                    pp = proj_ps.tile(
                        [128, 2 * BN], F32, tag="pp" + str(side), name="pp"
                    )
                    pp_v = pp.rearrange("p (c t) -> p c t", t=BN)
                    for fc in (2 * half, 2 * half + 1):
                        for pr in range(2):
                            _dr(
                                nc, pp_v[:, fc - 2 * half, :],
                                w_v[:, 2 * pr : 2 * pr + 2, fc * 128 : (fc + 1) * 128],
                                fusedT_p[:, 2 * pr : 2 * pr + 2, :],
                                start=(pr == 0), stop=(pr == 1),
                            )
                    # half-projection evacuation: q halves drain on ACT while
                    # PE runs k's matmuls and vice versa (ping-pong rings)
                    dst = psb_v[:, 2 * half : 2 * half + 2, :]
                    if COPY_LANE[side] == "act":
                        nc.scalar.activation(dst, pp_v[:], AF.Copy)
                    else:
                        nc.vector.tensor_copy(dst, pp_v[:])
                    sq_jobs.append((side, half, sqt_v, psb_v))
                sq_tiles[(j, side)] = sqt_v
            # squares read the SBUF fp8 copies (off the PE critical loop)
            for side, half, sqt_v, psb_v in sq_jobs:
                sdst = sqt_v[:, 2 * half : 2 * half + 2, :]
                ssrc = psb_v[:, 2 * half : 2 * half + 2, :]
                sq_lane = SQ_LANE[side][half]
                if sq_lane == "act":
                    nc.scalar.activation(sdst, ssrc, AF.Square)
                elif sq_lane == "dve":
                    nc.vector.tensor_mul(sdst, ssrc, ssrc)
                else:
                    nc.gpsimd.tensor_mul(sdst, ssrc, ssrc)

        invfs = {}

        def finish_group(g):
            S = len(GROUPS[g])
            n2 = n2_tiles[g]
            rsb = inv_pool.tile([2 * S, BN], F32, tag="rsb", name="rsb")
            nc.vector.reciprocal(rsb[:], n2[0 : 2 * S, :])
            inv = inv_pool.tile([2 * S, BN], BF16, tag="inv", name="inv")
            nc.scalar.activation(inv[:], rsb[:], AF.Sqrt)
            invf = inv_pool.tile([1, 2 * S * BN], BF16, tag="invf", name="invf")
            nc.sync.dma_start(invf[:], inv[:])
            invfs[g] = invf

        def tail(j):
            g, jg = GRP_OF[j], JG_OF[j]
            S = len(GROUPS[g])
            invf = invfs[g]
            qsb = qsbs.pop(j).rearrange("p (c t) -> p c t", t=BN)
            ksb = ksbs.pop(j).rearrange("p (c t) -> p c t", t=BN)
            lg = lg_ps.tile([128, BN], F32, tag="lg")
            ou = ou_ps.tile([128, BN], F32, tag="ou")
            for b in range(BL):
                bs = slice(b * 128, (b + 1) * 128)
                for pr in range(2):
                    _dr(
                        nc, lg[:, bs],
                        qsb[:, 2 * pr : 2 * pr + 2, bs],
                        ksb[:, 2 * pr : 2 * pr + 2, bs],
                        start=(pr == 0), stop=(pr == 1),
                    )
                qoff = jg * BN + b * 128
                koff = S * BN + jg * BN + b * 128
                nc.tensor.matmul(
                    ou[:, bs],
                    invf[:, qoff : qoff + 128],
                    invf[:, koff : koff + 128],
                    start=True, stop=True, skip_group_check=True,
                )
            # DVE may read only one PSUM operand; stage ou in SBUF with the
            # softmax 1/sqrt(D) folded into the copy scale
            ou_sb = sm_pool.tile([128, BN], BF16, tag="ou_sb")
            nc.scalar.activation(ou_sb[:], ou[:], AF.Copy, scale=EXP_SCALE)
            scb = sm_pool.tile([128, BN], BF16, tag="scb")
            nc.vector.tensor_mul(scb[:], lg[:], ou_sb[:])
            yb = sm_pool.tile([128, BN], BF16, tag="yb")
            # y = x + x^2/2 = exp(x)-1 to O(x^3); host adds the 1 and
            # normalizes (|x| <= 0.05 so rel err ~ 2e-5)
            acc = sm_pool.tile([128, 1], F32, tag="acc")
            nc.vector.affine_mul_reduce(
                yb[:], acc[:], scb[:], scb[:], scale=0.5, bias=1.0,
            )
            nc.sync.dma_start(ex_out[j].rearrange("n t -> n t"), yb[:])

        # software-pipelined schedule: norm matmuls lag production by one
        # basis; tails start as soon as their group's inverse norms exist.
        ablate = int(os.environ.get("KABLATE", "5"))
        if ablate < 5:
            if ablate >= 1:
                for j in range(K):
                    produce(j)
            if ablate >= 2:
                for j in range(K):
                    norm_mm(j)
            if ablate >= 3:
                for g in range(K // GRP):
                    finish_group(g)
            if ablate >= 4:
                for j in range(K):
                    tail(j)
            return
        produce(0)
        produce(1)
        norm_mm(0)
        produce(2)
        norm_mm(1)
        finish_group(0)
        produce(3)
        norm_mm(2)
        produce(4)
        norm_mm(3)
        finish_group(1)
        tail(0)
        produce(5)
        norm_mm(4)
        tail(1)
        produce(6)
        norm_mm(5)
        finish_group(2)
        tail(2)
        produce(7)
        norm_mm(6)
        finish_group(3)
        tail(3)
        tail(4)
        norm_mm(7)
        finish_group(4)
        tail(5)
        tail(6)
        tail(7)


_CACHE = {}


def _get_nc():
    if "nc" not in _CACHE:
        _CACHE["nc"] = build_kernel()
    return _CACHE["nc"]


def shard_inputs(desc_embeddings, name_value_embeddings, W_q, W_k, fusion_w, fusion_b):
    import ml_dtypes

    fp8 = ml_dtypes.float8_e4m3

    def to8(x):
        return np.ascontiguousarray(
            np.clip(np.asarray(x, dtype=np.float32), -224.0, 224.0).astype(fp8)
        )

    # fwT[p, ch, f] = fusion_w[f, ch*128+p]
    fwT = to8(
        np.asarray(fusion_w, np.float32)
        .T.reshape(CC, 128, D)
        .transpose(1, 0, 2)
        .reshape(128, CC * D)
    )
    # wq[j, p, dch, f] = W[j, dch*128+p, f]
    def wprep(W):
        return to8(
            np.asarray(W, np.float32)
            .reshape(K, DC, 128, D)
            .transpose(0, 2, 1, 3)
            .reshape(K, 128, DC * D)
        )

    wqk8 = np.concatenate([wprep(W_q), wprep(W_k)], axis=2)

    # one-hot scatter weights for the norm reduce: blocks 0-3 for
    # two-basis groups (col jg + 2*side), blocks 4-5 for singletons
    ohnp = np.zeros((128, 6 * 32), np.float32)
    for jg in range(2):
        for s in range(2):
            blk = (jg * 2 + s) * 32
            tgt = jg + 2 * s
            ohnp[:, blk + tgt] = 1.0
            ohnp[:, blk + 16 + tgt] = 1.0
    for s in range(2):
        blk = (4 + s) * 32
        ohnp[:, blk + s] = 1.0
        ohnp[:, blk + 16 + s] = 1.0
    oh8 = to8(ohnp)

    fb128 = np.ascontiguousarray(
        np.asarray(fusion_b, np.float32).reshape(DC, 128).T
    )

    desc = np.asarray(desc_embeddings, np.float32)
    nv = np.asarray(name_value_embeddings, np.float32)
    in_maps = []
    for c in range(CORES):
        dloc = desc[c * BL : (c + 1) * BL]      # [BL, N, D]
        nloc = nv[c * BL : (c + 1) * BL]
        # concatT[p, ch, b*128+n] = concat[b, n, ch*128+?]: chunks 0-3 desc,
        # 4-7 nv; within chunk partition p = feature ch*128+p
        cat = np.concatenate([dloc, nloc], axis=2)        # [BL, N, 2D]
        cT = (
            cat.reshape(BL * N, CC, 128)
            .transpose(2, 1, 0)                           # [128, CC, BL*N]
            .reshape(128, CC * BN)
        )
        in_maps.append(
            {
                "concatT": to8(cT),
                "fwT": fwT,
                "wqk": wqk8,
                "oh": oh8,
                "fb": fb128,
            }
        )
    return in_maps


def assemble_outputs(results):
    alpha = np.empty((B, K, N, N), dtype=np.float32)
    asum = np.zeros((K, N, N), dtype=np.float32)
    for c, r in enumerate(results):
        y = np.asarray(r["ex_out"], dtype=np.float32)     # [K, N, BL*N]
        y = y.reshape(K, N, BL, N)
        den = y.sum(axis=3, keepdims=True) + np.float32(N)
        al = (1.0 + y) / den                              # [K, N, BL, N]
        al = al.transpose(2, 0, 1, 3)                     # [BL, K, N, N]
        alpha[c * BL : (c + 1) * BL] = al
        asum += al.sum(axis=0)
    ema = np.float32(1.0 - MOMENTUM) * (asum / np.float32(B))
    bias_log = np.log(np.maximum(ema, np.float32(EPS)))
    bias_log = np.broadcast_to(bias_log[None], (B, K, N, N))
    return bias_log, alpha


def kernel(desc_embeddings, name_value_embeddings, W_q, W_k, fusion_w, fusion_b,
           _trace=False):
    nc = _get_nc()
    in_maps = shard_inputs(
        desc_embeddings, name_value_embeddings, W_q, W_k, fusion_w, fusion_b
    )
    res = run_bass_kernel_spmd(nc, in_maps, core_ids=list(range(CORES)), trace=_trace)
    out = assemble_outputs(res.results)
    if _trace:
        return out, res
    return out


# revision 28
# speedup vs baseline: 1.6539x; 1.0036x over previous
"""Trainium2 Bass kernel for BasisAffinityGAT (8-core data-parallel over batch).

Computation per batch b:
  fused = concat(desc, nv) @ fusion_w.T + fusion_b          [N, D]
  q_k = l2norm(fused @ W_q[k]); k_k = l2norm(fused @ W_k[k])
  alpha[b,k] = softmax(q_k @ k_k.T / sqrt(D))               [K, N, N]
Outputs: (bias_log, alpha); bias_log = log(max(0.01*mean_b(alpha), 1e-6)).

Strategy (fp8 DoubleRow, ~1.65x over the bf16 baseline):
- Batch sharded 4-per-core; all matmul operands cast to fp8-e4m3 on the host
  in pre-transposed, chunk-major layouts so the device does ZERO transposes.
- Every big contraction runs as fp8 DoubleRow matmuls (2 fp8/cell; pairs of
  128-row chunks contract per instruction at 0.5 cycles/row).
- Projections land in PSUM f32 as half-projections and are evacuated to SBUF
  fp8 by ACT/DVE copies in ping-pong rings (one PSUM tag per side) so the PE
  never waits long; squares run off the SBUF copies on Pool/ACT/DVE lanes.
- Norms: one-hot DoubleRow matmuls scatter per-basis sum(q^2)/sum(k^2) rows
  into a shared PSUM tile per norm group; one DVE reciprocal + one ACT Sqrt
  per group, then one SBUF->SBUF DMA flattens the inverse norms onto
  partition 0 to feed rank-1 outer-product matmuls. The last two bases form
  singleton groups so the pipeline drain is short.
- Softmax tail per basis: logits (fp8 DR) -> sc = lg * outer(invq,invk) on
  DVE (outer staged through SBUF with 1/sqrt(D) folded into the copy) ->
  y = sc + sc^2/2 (one custom DVE affine op; equals exp(sc)-1 to 2e-5 since
  |sc| <= 0.05). y ships in bf16; the host adds 1, divides by row sums
  (exact softmax), and finishes the EMA mean.
- ACT only ever uses {Copy, Identity, Square, Sqrt} = one activation table
  set (a dummy Sqrt pins it at t=0), so no 1.3us table reloads.
All fp8 rounding lands either ahead of the l2 normalization (which cancels
scale errors) or on cosine-scaled logits |x|<=0.05; measured rel err ~9e-4.
"""

import math
import os
import sys

import numpy as np

if "axon" not in os.environ.get("JAX_PLATFORMS", "axon"):
    os.environ.pop("JAX_PLATFORMS", None)

try:
    import concourse  # noqa: F401
except ImportError:  # pragma: no cover
    sys.path.insert(0, "/opt/trn_rl_repo")

import concourse.tile as tile  # noqa: E402
from concourse import bacc, mybir  # noqa: E402
from concourse.bass_utils import run_bass_kernel_spmd  # noqa: E402

B, N, D, K = 32, 128, 512, 8
CORES = 8
BL = B // CORES          # 4 local batches
BN = BL * N              # 512 tokens per core
DC = D // 128            # 4 feature chunks
CC = 2 * D // 128        # 8 concat chunks
GROUPS = [(0, 1), (2, 3), (4, 5), (6,), (7,)]   # norm-group partition
GRP_OF = {j: g for g, grp in enumerate(GROUPS) for j in grp}
JG_OF = {j: grp.index(j) for grp in GROUPS for j in grp}
GRP = 2                  # max bases per norm group (oh layout block size)
MOMENTUM = 0.99
EPS = 1e-6

F32 = mybir.dt.float32
BF16 = mybir.dt.bfloat16
FP8 = mybir.dt.float8e4
AF = mybir.ActivationFunctionType
PM = mybir.MatmulPerfMode

EXP_SCALE = 1.0 / math.sqrt(D)

# engine lane tables (tuned against TimelineSim): PSUM->SBUF copies per
# half-projection [side]; squares per half [side][half]
COPY_LANE = ["dve", "act"]
SQ_LANE = [["pool", "pool"], ["act", "dve"]]


def build_kernel():
    nc = bacc.Bacc(
        "TRN2",
        target_bir_lowering=False,
        debug=False,
        enable_asserts=False,
    )

    concatT = nc.dram_tensor("concatT", [128, CC * BN], FP8, kind="ExternalInput").ap()
    fwT = nc.dram_tensor("fwT", [128, CC * D], FP8, kind="ExternalInput").ap()
    wqk = nc.dram_tensor("wqk", [K, 128, 2 * DC * D], FP8, kind="ExternalInput").ap()
    oh = nc.dram_tensor("oh", [128, 6 * 32], FP8, kind="ExternalInput").ap()
    fb = nc.dram_tensor("fb", [128, DC], F32, kind="ExternalInput").ap()
    ex_out = nc.dram_tensor("ex_out", [K, N, BN], BF16, kind="ExternalOutput").ap()

    with tile.TileContext(nc) as tc:
        _emit(tc, concatT, fwT, wqk, oh, fb, ex_out)
    nc.finalize()
    return nc


def _dr(nc, out, lhsT, rhs, start, stop):
    nc.tensor.matmul(
        out, lhsT, rhs, start=start, stop=stop,
        perf_mode=PM.DoubleRow, skip_group_check=True,
    )


def _emit(tc, concatT, fwT, wqk, oh, fb, ex_out):
    nc = tc.nc
    from contextlib import ExitStack

    ctx = ExitStack()
    with ctx:
        const_pool = ctx.enter_context(tc.tile_pool(name="const", bufs=1))
        fused_pool = ctx.enter_context(tc.tile_pool(name="fused", bufs=1))
        w_pool = ctx.enter_context(tc.tile_pool(name="w", bufs=4))
        qk_pool = ctx.enter_context(tc.tile_pool(name="qk", bufs=8))
        sq_pool = ctx.enter_context(tc.tile_pool(name="sq", bufs=4))
        inv_pool = ctx.enter_context(tc.tile_pool(name="inv", bufs=3))
        sm_pool = ctx.enter_context(tc.tile_pool(name="sm", bufs=4))
        proj_ps = ctx.enter_context(tc.tile_pool(name="proj_ps", bufs=1, space="PSUM"))
        n2_ps = ctx.enter_context(tc.tile_pool(name="n2_ps", bufs=2, space="PSUM"))
        lg_ps = ctx.enter_context(tc.tile_pool(name="lg_ps", bufs=1, space="PSUM"))
        ou_ps = ctx.enter_context(tc.tile_pool(name="ou_ps", bufs=1, space="PSUM"))

        # ---- constants ---------------------------------------------------
        # tiny dummy Sqrt pins the {sqrt, square, copy, identity} ACT table
        # set once at t=0; every later activation stays in-set (no reloads)
        dummy = const_pool.tile([1, 1], F32)
        nc.vector.memset(dummy[:], 1.0)
        nc.scalar.activation(dummy[:], dummy[:], AF.Sqrt)
        oh_sb = const_pool.tile([128, 6 * 32], FP8)
        nc.sync.dma_start(oh_sb[:], oh)
        fb_sb = const_pool.tile([128, DC], F32)
        nc.sync.dma_start(fb_sb[:], fb)

        # ---- fused = concat @ fw.T + fb, in fp8 transposed layout --------
        # fusedT[p, f, t] over f-chunks; contraction over CC=8 chunks as 4
        # DoubleRow pairs.
        fusedT = fused_pool.tile([128, DC * BN], FP8)
        fusedT_v = fusedT.rearrange("p (c t) -> p c t", t=BN)
        with tc.tile_pool(name="prep", bufs=1) as prep_pool:
            ccT = prep_pool.tile([128, CC * BN], FP8)
            fwT_sb = prep_pool.tile([128, CC * D], FP8)
            half = CC // 2
            nc.sync.dma_start(fwT_sb[:, : half * D], fwT[:, : half * D])
            nc.sync.dma_start(ccT[:, : half * BN], concatT[:, : half * BN])
            nc.sync.dma_start(fwT_sb[:, half * D :], fwT[:, half * D :])
            nc.sync.dma_start(ccT[:, half * BN :], concatT[:, half * BN :])
            ccT_v = ccT.rearrange("p (c t) -> p c t", t=BN)
            fwT_v = fwT_sb.rearrange("p (c f) -> p c f", f=D)
            for f in range(DC):
                fps = proj_ps.tile([128, BN], F32, tag="pp0", name="fps")
                for pr in range(CC // 2):
                    _dr(
                        nc, fps[:],
                        fwT_v[:, 2 * pr : 2 * pr + 2, f * 128 : (f + 1) * 128],
                        ccT_v[:, 2 * pr : 2 * pr + 2, :],
                        start=(pr == 0), stop=(pr == CC // 2 - 1),
                    )
                if f % 2 == 0:
                    nc.scalar.activation(
                        fusedT_v[:, f, :], fps[:],
                        AF.Identity, bias=fb_sb[:, f : f + 1],
                    )
                else:
                    nc.vector.tensor_scalar_add(
                        fusedT_v[:, f, :], fps[:], fb_sb[:, f : f + 1],
                    )

        fusedT_p = fusedT.rearrange("p (c t) -> p c t", t=BN)

        # ---- per-basis production + grouped norm / softmax tail ----------
        # Per group of 4 bases, n2[16, BN] collects row jg (q) / 4+jg (k).
        n2_tiles = {}
        qsbs, ksbs = {}, {}
        sq_tiles = {}

        def norm_mm(j):
            # one-hot DoubleRow matmuls into n2 rows (row jg for q, S+jg
            # for k; other rows take zeros). Emitted one basis late so PE
            # never head-of-line blocks on the squares.
            g, jg = GRP_OF[j], JG_OF[j]
            S = len(GROUPS[g])
            n2 = n2_tiles[g]
            for side in range(2):
                sqt_v = sq_tiles.pop((j, side))
                if S == 2:
                    blk = (jg * 2 + side) * 32
                else:
                    blk = (4 + side) * 32
                ohv = oh_sb[:, blk : blk + 32].rearrange(
                    "p (i c) -> p i c", i=2
                )
                first = jg == 0 and side == 0
                last = jg == S - 1 and side == 1
                for pr in range(2):
                    _dr(
                        nc, n2[:],
                        ohv[:],
                        sqt_v[:, 2 * pr : 2 * pr + 2, :],
                        start=(first and pr == 0), stop=(last and pr == 1),
                    )

        def produce(j):
            g, jg = GRP_OF[j], JG_OF[j]
            wqk_sb = w_pool.tile([128, 2 * DC * D], FP8, tag="wqk")
            nc.sync.dma_start(wqk_sb[:], wqk[j])
            wq_sb = wqk_sb[:, : DC * D]
            wk_sb = wqk_sb[:, DC * D :]

            if jg == 0:
                n2_tiles[g] = n2_ps.tile([16, BN], F32, tag="n2", name="n2")
            n2 = n2_tiles[g]
            del n2

            sq_jobs = []
            for side, (w_sb, store) in enumerate(
                ((wq_sb, qsbs), (wk_sb, ksbs))
            ):
                w_v = w_sb.rearrange("p (c f) -> p c f", f=D)
                psb = qk_pool.tile([128, DC * BN], FP8, tag="qk" + str(side), name="psb")
                store[j] = psb
                sqt = sq_pool.tile([128, DC * BN], FP8, tag="sq" + str(side), name="sqt")
                psb_v = psb.rearrange("p (c t) -> p c t", t=BN)
                sqt_v = sqt.rearrange("p (c t) -> p c t", t=BN)
                for half in range(2):
                    pp = proj_ps.tile(
                        [128, 2 * BN], F32, tag="pp" + str(side), name="pp"
                    )
                    pp_v = pp.rearrange("p (c t) -> p c t", t=BN)
                    for fc in (2 * half, 2 * half + 1):
                        for pr in range(2):
                            _dr(
                                nc, pp_v[:, fc - 2 * half, :],
                                w_v[:, 2 * pr : 2 * pr + 2, fc * 128 : (fc + 1) * 128],
                                fusedT_p[:, 2 * pr : 2 * pr + 2, :],
                                start=(pr == 0), stop=(pr == 1),
                            )
                    # half-projection evacuation: q halves drain on ACT while
                    # PE runs k's matmuls and vice versa (ping-pong rings)
                    dst = psb_v[:, 2 * half : 2 * half + 2, :]
                    if COPY_LANE[side] == "act":
                        nc.scalar.activation(dst, pp_v[:], AF.Copy)
                    else:
                        nc.vector.tensor_copy(dst, pp_v[:])
                    sq_jobs.append((side, half, sqt_v, psb_v))
                sq_tiles[(j, side)] = sqt_v
            # squares read the SBUF fp8 copies (off the PE critical loop)
            for side, half, sqt_v, psb_v in sq_jobs:
                sdst = sqt_v[:, 2 * half : 2 * half + 2, :]
                ssrc = psb_v[:, 2 * half : 2 * half + 2, :]
                sq_lane = SQ_LANE[side][half]
                if sq_lane == "act":
                    nc.scalar.activation(sdst, ssrc, AF.Square)
                elif sq_lane == "dve":
                    nc.vector.tensor_mul(sdst, ssrc, ssrc)
                else:
                    nc.gpsimd.tensor_mul(sdst, ssrc, ssrc)

        invfs = {}

        def finish_group(g):
            S = len(GROUPS[g])
            n2 = n2_tiles[g]
            rsb = inv_pool.tile([2 * S, BN], F32, tag="rsb", name="rsb")
            nc.vector.reciprocal(rsb[:], n2[0 : 2 * S, :])
            inv = inv_pool.tile([2 * S, BN], BF16, tag="inv", name="inv")
            nc.scalar.activation(inv[:], rsb[:], AF.Sqrt)
            invf = inv_pool.tile([1, 2 * S * BN], BF16, tag="invf", name="invf")
            nc.sync.dma_start(invf[:], inv[:])
            invfs[g] = invf

        def tail(j):
            g, jg = GRP_OF[j], JG_OF[j]
            S = len(GROUPS[g])
            invf = invfs[g]
            qsb = qsbs.pop(j).rearrange("p (c t) -> p c t", t=BN)
            ksb = ksbs.pop(j).rearrange("p (c t) -> p c t", t=BN)
            lg = lg_ps.tile([128, BN], F32, tag="lg")
            ou = ou_ps.tile([128, BN], F32, tag="ou")
            for b in range(BL):
                bs = slice(b * 128, (b + 1) * 128)
                for pr in range(2):
                    _dr(
                        nc, lg[:, bs],
                        qsb[:, 2 * pr : 2 * pr + 2, bs],
                        ksb[:, 2 * pr : 2 * pr + 2, bs],
                        start=(pr == 0), stop=(pr == 1),
                    )
                qoff = jg * BN + b * 128
                koff = S * BN + jg * BN + b * 128
                nc.tensor.matmul(
                    ou[:, bs],
                    invf[:, qoff : qoff + 128],
                    invf[:, koff : koff + 128],
                    start=True, stop=True, skip_group_check=True,
                )
            # DVE may read only one PSUM operand; stage ou in SBUF with the
            # softmax 1/sqrt(D) folded into the copy scale
            ou_sb = sm_pool.tile([128, BN], BF16, tag="ou_sb")
            nc.scalar.activation(ou_sb[:], ou[:], AF.Copy, scale=EXP_SCALE)
            scb = sm_pool.tile([128, BN], BF16, tag="scb")
            nc.vector.tensor_mul(scb[:], lg[:], ou_sb[:])
            yb = sm_pool.tile([128, BN], BF16, tag="yb")
            # y = x + x^2/2 = exp(x)-1 to O(x^3); host adds the 1 and
            # normalizes (|x| <= 0.05 so rel err ~ 2e-5)
            acc = sm_pool.tile([128, 1], F32, tag="acc")
            nc.vector.affine_mul_reduce(
                yb[:], acc[:], scb[:], scb[:], scale=0.5, bias=1.0,
            )
            nc.sync.dma_start(ex_out[j].rearrange("n t -> n t"), yb[:])

        # software-pipelined schedule: norm matmuls lag production by one
        # basis; tails start as soon as their group's inverse norms exist.
        produce(0)
        produce(1)
        produce(2)
        norm_mm(0)
        produce(3)
        norm_mm(1)
        finish_group(0)
        produce(4)
        norm_mm(2)
        tail(0)
        produce(5)
        norm_mm(3)
        finish_group(1)
        tail(1)
        produce(6)
        norm_mm(4)
        tail(2)
        produce(7)
        norm_mm(5)
        finish_group(2)
        tail(3)
        norm_mm(6)
        finish_group(3)
        tail(4)
        tail(5)
        norm_mm(7)
        finish_group(4)
        tail(6)
        tail(7)


_CACHE = {}


def _get_nc():
    if "nc" not in _CACHE:
        _CACHE["nc"] = build_kernel()
    return _CACHE["nc"]


def shard_inputs(desc_embeddings, name_value_embeddings, W_q, W_k, fusion_w, fusion_b):
    import ml_dtypes

    fp8 = ml_dtypes.float8_e4m3

    def to8(x):
        return np.ascontiguousarray(
            np.clip(np.asarray(x, dtype=np.float32), -224.0, 224.0).astype(fp8)
        )

    # fwT[p, ch, f] = fusion_w[f, ch*128+p]
    fwT = to8(
        np.asarray(fusion_w, np.float32)
        .T.reshape(CC, 128, D)
        .transpose(1, 0, 2)
        .reshape(128, CC * D)
    )
    # wq[j, p, dch, f] = W[j, dch*128+p, f]
    def wprep(W):
        return to8(
            np.asarray(W, np.float32)
            .reshape(K, DC, 128, D)
            .transpose(0, 2, 1, 3)
            .reshape(K, 128, DC * D)
        )

    wqk8 = np.concatenate([wprep(W_q), wprep(W_k)], axis=2)

    # one-hot scatter weights for the norm reduce: blocks 0-3 for
    # two-basis groups (col jg + 2*side), blocks 4-5 for singletons
    ohnp = np.zeros((128, 6 * 32), np.float32)
    for jg in range(2):
        for s in range(2):
            blk = (jg * 2 + s) * 32
            tgt = jg + 2 * s
            ohnp[:, blk + tgt] = 1.0
            ohnp[:, blk + 16 + tgt] = 1.0
    for s in range(2):
        blk = (4 + s) * 32
        ohnp[:, blk + s] = 1.0
        ohnp[:, blk + 16 + s] = 1.0
    oh8 = to8(ohnp)

    fb128 = np.ascontiguousarray(
        np.asarray(fusion_b, np.float32).reshape(DC, 128).T
    )

    desc = np.asarray(desc_embeddings, np.float32)
    nv = np.asarray(name_value_embeddings, np.float32)
    in_maps = []
    for c in range(CORES):
        dloc = desc[c * BL : (c + 1) * BL]      # [BL, N, D]
        nloc = nv[c * BL : (c + 1) * BL]
        # concatT[p, ch, b*128+n] = concat[b, n, ch*128+?]: chunks 0-3 desc,
        # 4-7 nv; within chunk partition p = feature ch*128+p
        cat = np.concatenate([dloc, nloc], axis=2)        # [BL, N, 2D]
        cT = (
            cat.reshape(BL * N, CC, 128)
            .transpose(2, 1, 0)                           # [128, CC, BL*N]
            .reshape(128, CC * BN)
        )
        in_maps.append(
            {
                "concatT": to8(cT),
                "fwT": fwT,
                "wqk": wqk8,
                "oh": oh8,
                "fb": fb128,
            }
        )
    return in_maps


def assemble_outputs(results):
    alpha = np.empty((B, K, N, N), dtype=np.float32)
    asum = np.zeros((K, N, N), dtype=np.float32)
    for c, r in enumerate(results):
        y = np.asarray(r["ex_out"], dtype=np.float32)     # [K, N, BL*N]
        y = y.reshape(K, N, BL, N)
        den = y.sum(axis=3, keepdims=True) + np.float32(N)
        al = (1.0 + y) / den                              # [K, N, BL, N]
        al = al.transpose(2, 0, 1, 3)                     # [BL, K, N, N]
        alpha[c * BL : (c + 1) * BL] = al
        asum += al.sum(axis=0)
    ema = np.float32(1.0 - MOMENTUM) * (asum / np.float32(B))
    bias_log = np.log(np.maximum(ema, np.float32(EPS)))
    bias_log = np.broadcast_to(bias_log[None], (B, K, N, N))
    return bias_log, alpha


def kernel(desc_embeddings, name_value_embeddings, W_q, W_k, fusion_w, fusion_b,
           _trace=False):
    nc = _get_nc()
    in_maps = shard_inputs(
        desc_embeddings, name_value_embeddings, W_q, W_k, fusion_w, fusion_b
    )
    res = run_bass_kernel_spmd(nc, in_maps, core_ids=list(range(CORES)), trace=_trace)
    out = assemble_outputs(res.results)
    if _trace:
        return out, res
    return out
